# revision 1
# baseline (speedup 1.0000x reference)
"""MDLSTM cell (2-direction) Bass/Tile kernel for Trainium2, 8-core SPMD.

Math (per direction d, with shared input projections):
    i = sigmoid(w_ii @ x + w_hi @ h_d + b_i)
    f = sigmoid(w_if @ x + w_hf @ h_d + b_f)
    g = tanh   (w_ig @ x + w_hg @ h_d + b_g)
    o = sigmoid(w_io @ x + w_ho @ h_d + b_o)
    c_d = f * c_prev_d + i * g
    h_d = o * tanh(c_d)
ct = ws0 * c_0 + ws1 * c_1 ;  ht = ws0 * h_0 + ws1 * h_1

Sharding: all activations/states split along N (=8192) across 8 cores;
weights replicated. No cross-core communication.

Per-core kernel: per output row tile (M=128) the 4 shared input
projections are computed once into PSUM (start=True groups) and copied to
SBUF; each of the 8 gate/direction accumulations then starts by injecting
that x-projection into its PSUM bank via a VectorE copy and accumulates
the 8 hidden-projection K-tiles on top (start=False matmuls — PE-write
accumulate onto engine-written PSUM, valid because every bank's first
group in program order is a start=True group that defines has_written).
ScalarE applies sigmoid/tanh + per-partition bias straight out of PSUM;
VectorE does the elementwise cell update and direction combine. Matmul
operands use float32r (fp32 storage, single-pass reduced-precision PE
mode: bf16-class throughput at N>=256, ~1.5e-4 matmul rel err).
"""

import numpy as np

import concourse.bass as bass  # noqa: F401  (bass types via bacc/tile)
import concourse.mybir as mybir
import concourse.tile as tile
from concourse import bacc
from concourse.bass_utils import run_bass_kernel_spmd

N_CORES = 8
IN_C = 512
OUT_C = 1024
N = 8192
NS = N // N_CORES  # columns per core
NCH = 512  # psum free-dim chunk (one bank)
N_CHUNKS = NS // NCH
KX = IN_C // 128  # k-tiles of the input projection
KH = OUT_C // 128  # k-tiles of the hidden projection
M_TILES = OUT_C // 128

F32 = mybir.dt.float32
MM_MODE = "fp32r"  # one of: "fp32r", "bf16", "fp16"
import ml_dtypes as _mld
MM_DT = {"fp32r": mybir.dt.float32r, "bf16": mybir.dt.bfloat16,
         "fp16": mybir.dt.float16}[MM_MODE]
MM_NP = {"fp32r": np.float32, "bf16": _mld.bfloat16,
         "fp16": np.float16}[MM_MODE]

SIG = mybir.ActivationFunctionType.Sigmoid
TANH = mybir.ActivationFunctionType.Tanh
MULT = mybir.AluOpType.mult
ADD = mybir.AluOpType.add
COPY = mybir.ActivationFunctionType.Copy


def _build(ws0: float, ws1: float):
    nc = bacc.Bacc(
        "TRN2", target_bir_lowering=False, debug=False, num_devices=N_CORES
    )

    xd = nc.dram_tensor("x", [128, KX, NS], MM_DT, kind="ExternalInput")
    hd_ = [
        nc.dram_tensor(f"h{d}", [128, KH, NS], MM_DT, kind="ExternalInput")
        for d in (0, 1)
    ]
    cd_ = [
        nc.dram_tensor(f"c{d}", [OUT_C, NS], F32, kind="ExternalInput")
        for d in (0, 1)
    ]
    # weights: [gate, m_tile, partition(k%128), k_tile, m_in_tile]
    wxd = nc.dram_tensor("wx", [4, M_TILES, 128, KX, 128], MM_DT, kind="ExternalInput")
    whd = nc.dram_tensor("wh", [4, M_TILES, 128, KH, 128], MM_DT, kind="ExternalInput")
    biasd = nc.dram_tensor("bias", [128, 4 * M_TILES], F32, kind="ExternalInput")
    ctd = nc.dram_tensor("ct", [OUT_C, NS], F32, kind="ExternalOutput")
    htd = nc.dram_tensor("ht", [OUT_C, NS], F32, kind="ExternalOutput")

    with tile.TileContext(nc) as tc:
        with (
            tc.tile_pool(name="resident", bufs=1) as res_pool,
            tc.tile_pool(name="wx", bufs=8) as wx_pool,
            tc.tile_pool(name="wh", bufs=8) as wh_pool,
            tc.tile_pool(name="psum", bufs=8, space="PSUM") as ps_pool,
            tc.tile_pool(name="xproj", bufs=8) as xp_pool,
            tc.tile_pool(name="gates", bufs=6) as g_pool,
            tc.tile_pool(name="cprev", bufs=3) as cp_pool,
            tc.tile_pool(name="tmp", bufs=2) as t_pool,
            tc.tile_pool(name="dirres", bufs=4) as dr_pool,
            tc.tile_pool(name="out", bufs=2) as o_pool,
        ):
            wx_tiles: dict = {}
            wh_tiles: dict = {}

            def alloc_w(mt):
                wx_tiles[mt] = [
                    wx_pool.tile([128, KX, 128], MM_DT, tag="wx", name=f"wx_{mt}_{g}")
                    for g in range(4)
                ]
                wh_tiles[mt] = [
                    wh_pool.tile([128, KH, 128], MM_DT, tag="wh", name=f"wh_{mt}_{g}")
                    for g in range(4)
                ]

            def load_w(mt):
                alloc_w(mt)
                for g in range(4):
                    nc.sync.dma_start(wx_tiles[mt][g][:], wxd[g, mt])
                    nc.sync.dma_start(wh_tiles[mt][g][:], whd[g, mt])

            x_sb = res_pool.tile([128, KX, NS], MM_DT, tag="x")
            h_sb = [
                res_pool.tile([128, KH, NS], MM_DT, tag=f"h{d}", name=f"h_sb{d}")
                for d in (0, 1)
            ]
            bias_sb = res_pool.tile([128, 4 * M_TILES], F32, tag="bias")

            # Startup is DMA-feed-limited (~9MB before the first d1 groups
            # at ~300GB/s); coarse transfers measured faster than
            # fine-grained ones (fewer descriptors contending with PE's
            # SBUF reads).
            nc.sync.dma_start(bias_sb[:], biasd[:])
            load_w(0)
            for n in range(N_CHUNKS):
                nsl = slice(n * NCH, (n + 1) * NCH)
                nc.sync.dma_start(x_sb[:, :, nsl], xd[:, :, nsl])
                nc.sync.dma_start(h_sb[0][:, :, nsl], hd_[0][:, :, nsl])
                nc.sync.dma_start(h_sb[1][:, :, nsl], hd_[1][:, :, nsl])
            load_w(1)

            def px_phase(mt, n, wxm):
                nsl = slice(n * NCH, (n + 1) * NCH)
                xp = []
                for g in range(4):
                    px = ps_pool.tile(
                        [128, NCH], F32, tag="ps", name=f"px_{mt}_{n}_{g}"
                    )
                    for kt in range(KX):
                        nc.tensor.matmul(
                            px[:],
                            wxm[g][:, kt, :],
                            x_sb[:, kt, nsl],
                            start=(kt == 0),
                            stop=(kt == KX - 1),
                        )
                    xpt = xp_pool.tile(
                        [128, NCH], F32, tag="xp", name=f"xp_{mt}_{n}_{g}"
                    )
                    nc.scalar.activation(xpt[:], px[:], COPY)
                    xp.append(xpt)
                return xp

            def dir_phase(mt, n, d, xp, whm, msl):
                nsl = slice(n * NCH, (n + 1) * NCH)
                gt = []
                for g in range(4):
                    ps = ps_pool.tile(
                        [128, NCH], F32, tag="ps", name=f"ps_{mt}_{n}_{d}_{g}"
                    )
                    # inject the shared x-projection, then accumulate the
                    # hidden projection on top of it
                    nc.vector.tensor_copy(ps[:], xp[g][:])
                    for kh in range(KH):
                        nc.tensor.matmul(
                            ps[:],
                            whm[g][:, kh, :],
                            h_sb[d][:, kh, nsl],
                            start=False,
                            stop=(kh == KH - 1),
                            skip_group_check=True,
                        )
                    gact = g_pool.tile(
                        [128, NCH], F32, tag="gate", name=f"gate_{mt}_{n}_{d}_{g}"
                    )
                    nc.scalar.activation(
                        gact[:],
                        ps[:],
                        TANH if g == 2 else SIG,
                        bias=bias_sb[:, g * M_TILES + mt : g * M_TILES + mt + 1],
                    )
                    gt.append(gact)

                cp = cp_pool.tile([128, NCH], F32, tag="cp")
                nc.sync.dma_start(cp[:], cd_[d][msl, nsl])
                ig = t_pool.tile([128, NCH], F32, tag="ig")
                nc.vector.tensor_mul(ig[:], gt[0][:], gt[2][:])
                fc = t_pool.tile([128, NCH], F32, tag="fc")
                nc.vector.tensor_mul(fc[:], gt[1][:], cp[:])
                cnew = dr_pool.tile([128, NCH], F32, tag="cnew")
                nc.vector.tensor_add(cnew[:], ig[:], fc[:])
                tch = t_pool.tile([128, NCH], F32, tag="tch")
                nc.scalar.activation(tch[:], cnew[:], TANH)
                hnew = dr_pool.tile([128, NCH], F32, tag="hnew")
                nc.vector.tensor_mul(hnew[:], gt[3][:], tch[:])
                return cnew, hnew

            def combine(n, msl, cdir, hdir):
                nsl = slice(n * NCH, (n + 1) * NCH)
                c0s = t_pool.tile([128, NCH], F32, tag="c0s")
                nc.vector.tensor_scalar_mul(c0s[:], cdir[0][:], ws0)
                ctt = o_pool.tile([128, NCH], F32, tag="ctt")
                nc.vector.scalar_tensor_tensor(
                    ctt[:], cdir[1][:], ws1, c0s[:], MULT, ADD
                )
                nc.sync.dma_start(ctd[msl, nsl], ctt[:])
                h0s = t_pool.tile([128, NCH], F32, tag="h0s")
                nc.vector.tensor_scalar_mul(h0s[:], hdir[0][:], ws0)
                htt = o_pool.tile([128, NCH], F32, tag="htt")
                nc.vector.scalar_tensor_tensor(
                    htt[:], hdir[1][:], ws1, h0s[:], MULT, ADD
                )
                nc.sync.dma_start(htd[msl, nsl], htt[:])

            for mt in range(M_TILES):
                msl = slice(mt * 128, (mt + 1) * 128)
                if mt + 2 < M_TILES:
                    load_w(mt + 2)
                wxm = wx_tiles.pop(mt)
                whm = wh_tiles.pop(mt)

                # Both n-chunks' input projections first: at kernel start
                # these 8 start=True groups cover all 8 PSUM banks, so no
                # inject group ever runs on a virgin bank with undefined
                # has_written (accumulate-vs-overwrite) state.
                xp0 = px_phase(mt, 0, wxm)
                xp1 = px_phase(mt, 1, wxm)
                c00, h00 = dir_phase(mt, 0, 0, xp0, whm, msl)
                c10, h10 = dir_phase(mt, 0, 1, xp0, whm, msl)
                combine(0, msl, [c00, c10], [h00, h10])
                c01, h01 = dir_phase(mt, 1, 0, xp1, whm, msl)
                c11, h11 = dir_phase(mt, 1, 1, xp1, whm, msl)
                combine(1, msl, [c01, c11], [h01, h11])

    nc.finalize()
    n_mm = sum(
        1 for i in nc.inst_map.values() if type(i).__name__ == "InstMatmult"
    )
    expected_mm = M_TILES * N_CHUNKS * 4 * (KX + 2 * KH)
    assert n_mm == expected_mm, f"matmul count {n_mm} != {expected_mm}"
    return nc


_CACHE: dict = {}


def _get_nc(ws0: float, ws1: float):
    key = (ws0, ws1)
    if key not in _CACHE:
        _CACHE.clear()
        _CACHE[key] = _build(ws0, ws1)
    return _CACHE[key]


def _prep_w(w: np.ndarray, kt: int) -> np.ndarray:
    """(OUT_C, K) weight -> [m_tile, partition, k_tile, m_in_tile] lhsT tiles."""
    wT = np.ascontiguousarray(w.T)  # (K, OUT_C)
    k = wT.shape[0]
    assert k == kt * 128
    r = wT.reshape(kt, 128, M_TILES, 128)  # [ktile, p, mtile, mi]
    return np.ascontiguousarray(r.transpose(2, 1, 0, 3).astype(MM_NP))


def _prep_rhs(a: np.ndarray, kt: int) -> np.ndarray:
    """(K, n) activation -> [partition, k_tile, n]."""
    k, n = a.shape
    assert k == kt * 128
    return np.ascontiguousarray(a.reshape(kt, 128, n).transpose(1, 0, 2).astype(MM_NP))


def run(inputs: dict, trace: bool = False, trace_kwargs: dict | None = None):
    x = np.asarray(inputs["x"], dtype=np.float32)
    ws = np.asarray(inputs["weighted_sum"], dtype=np.float32)
    ws0, ws1 = float(ws[0]), float(ws[1])
    nc = _get_nc(ws0, ws1)

    wx_host = np.stack(
        [_prep_w(np.asarray(inputs[k], dtype=np.float32), KX)
         for k in ("w_ii", "w_if", "w_ig", "w_io")]
    )
    wh_host = np.stack(
        [_prep_w(np.asarray(inputs[k], dtype=np.float32), KH)
         for k in ("w_hi", "w_hf", "w_hg", "w_ho")]
    )
    bias_host = np.concatenate(
        [np.asarray(inputs[k], dtype=np.float32).reshape(M_TILES, 128).T
         for k in ("b_i", "b_f", "b_g", "b_o")],
        axis=1,
    )
    bias_host = np.ascontiguousarray(bias_host)

    h0 = np.asarray(inputs["h_prev_dim0"], dtype=np.float32)
    h1 = np.asarray(inputs["h_prev_dim1"], dtype=np.float32)
    c0 = np.asarray(inputs["c_prev_dim0"], dtype=np.float32)
    c1 = np.asarray(inputs["c_prev_dim1"], dtype=np.float32)

    in_maps = []
    for core in range(N_CORES):
        csl = slice(core * NS, (core + 1) * NS)
        in_maps.append(
            {
                "x": _prep_rhs(x[:, csl], KX),
                "h0": _prep_rhs(h0[:, csl], KH),
                "h1": _prep_rhs(h1[:, csl], KH),
                "c0": np.ascontiguousarray(c0[:, csl]),
                "c1": np.ascontiguousarray(c1[:, csl]),
                "wx": wx_host,
                "wh": wh_host,
                "bias": bias_host,
            }
        )

    res = run_bass_kernel_spmd(
        nc,
        in_maps,
        list(range(N_CORES)),
        trace=trace,
        **(trace_kwargs or {}),
    )
    ct = np.concatenate([res.results[c]["ct"] for c in range(N_CORES)], axis=1)
    ht = np.concatenate([res.results[c]["ht"] for c in range(N_CORES)], axis=1)
    return (ct, ht), res


def kernel(**inputs) -> tuple:
    (ct, ht), _ = run(inputs)
    return ct, ht

